# revision 30
# baseline (speedup 1.0000x reference)
"""BERT self-attention on 8 Trainium2 NeuronCores.

Sharding: data-parallel over batch (B=8 -> 1 batch element per core).
Every core runs the same single-core Bass kernel on its own batch slice;
weights/mask are replicated. The final output is a host-side stack.

Per-core algorithm (S=1024, HID=1024, NH=16, HD=64), all matmuls bf16
with fp32 PSUM accumulation:

  xT = X^T (host-transposed, bf16)             [HID, S]
  Q^T = Wq^T @ X^T   (lhsT = Wq natural)       [HID, S]  (+bq per-partition)
  K^T = Wk^T @ X^T                             [HID, S]  (+bk per-partition)
  V   = X @ Wv       (lhsT = xT)               [S, HID]  (+bv broadcast)
  per head h:
    S^T = K_h @ Q_h^T            (scoresT: [k, q]; K^T stored zero-padded to
                                  128 contraction rows so FWL stays enabled)
    P^T = exp(S^T/8 + mask[k])   (ScalarE, mask is per-partition in this layout,
                                  max-subtraction skipped: |scores/8| <~ 4)
    ctx = P^T.T @ [V_h + bv | 1] (lhsT = P^T directly, no transposes anywhere;
                                  the ones column yields the softmax denominator Z)
    out[:, h] = ctx[:, :64] * (1/Z)

v2 schedule (measured v1: PE busy 191us of 213us total at full clock; the
PE moving-row stream IS the bottleneck; exp stream 142us fits under it):

  - Input DMA issue is spread across Scalar/Sync/GpSimd sequencers in
    priority order (v1 serialized 19 issues x 0.72us on Sync; wq/wk only
    landed at 34us and the first exp fired at 50us).
  - Host repacks: wq+wk interleaved per 128-row chunk (one [128,2048] DMA
    lands both), wv in head-pair column blocks [blk][p][kc][col], biases
    and mask combined into one [128,24] tensor.
  - Fill phase: Q(c0)+K(c0) projections accumulate chunk-by-chunk as xT
    chunks land (4 open PSUM chains: Q halves in the qkv pool, K halves
    parked in the ctx pool which is idle during fill). Real work replaces
    v1's 28 warmup dummies; ~7 dummies only bridge t=6.5..8.7us.
  - Steady slots c=0..7 (heads a=2c, b=2c+1): per kt the two heads'
    score matmuls + exps are emitted as a pair, followed by a filler
    quota drawn from this slot's work list (next-chunk QK projection
    groups, V column-block st-chains, one-slot-lagged ctx chains with
    the rotated accumulation order). Pairs cadence ~2.4us > exp-pair
    2.2us, so neither the 2-buf score PSUM pool nor ScalarE stalls.
  - ctx lags its slot by one (pT pool bufs=4), so every ctx chain's exps
    are complete when emitted; per-head output leaves as one 3D-AP DMA.
"""

import functools

import numpy as np
import ml_dtypes

B, S, HID = 8, 1024, 1024
NH, HD = 16, 64
P = 128
NCH = HID // P  # hid chunks (8)
NKT = S // P  # key tiles (8)
NQT = S // P  # query tiles (8)
VROW = NH * (HD + 1)  # 1040: per-seq-chunk V row: 16 x (64 V cols + ones col)
N_CORES = 8

SCALE = 1.0 / float(np.sqrt(HD))


@functools.lru_cache(maxsize=None)
def _build(has_bv: bool):
    import concourse.bass as bass
    import concourse.tile as tile
    from concourse import bacc, mybir
    from contextlib import ExitStack

    fp32 = mybir.dt.float32
    bf16 = mybir.dt.bfloat16
    EXP = mybir.ActivationFunctionType.Exp

    nc = bacc.Bacc("TRN2", target_bir_lowering=False)

    xT = nc.dram_tensor("xT", [HID, S], bf16, kind="ExternalInput")
    wqk = nc.dram_tensor("wqk", [HID, 2 * HID], bf16, kind="ExternalInput")
    # wv repacked on host as [head-pair block][partition][chunk][col]
    wvs = nc.dram_tensor("wvs", [NCH, P, NCH, P], bf16, kind="ExternalInput")
    bqkm = nc.dram_tensor("bqkm", [P, 3 * NCH], fp32, kind="ExternalInput")
    bv = nc.dram_tensor("bv", [HID], fp32, kind="ExternalInput") if has_bv else None
    out = nc.dram_tensor("out", [S, HID], fp32, kind="ExternalOutput")

    with tile.TileContext(nc) as tc, ExitStack() as ctx:
        persist = ctx.enter_context(tc.tile_pool(name="persist", bufs=1))
        misc = ctx.enter_context(tc.tile_pool(name="misc", bufs=8))
        pT_pool = ctx.enter_context(tc.tile_pool(name="pT", bufs=4))
        out_pool = ctx.enter_context(tc.tile_pool(name="out", bufs=4))
        qkv_ps = ctx.enter_context(tc.tile_pool(name="qkv_ps", bufs=2, space="PSUM"))
        sc_ps = ctx.enter_context(tc.tile_pool(name="sc_ps", bufs=2, space="PSUM"))
        cx_ps = ctx.enter_context(tc.tile_pool(name="cx_ps", bufs=2, space="PSUM"))

        # ---- persistent SBUF tensors ----
        xT_c = [persist.tile([P, S], bf16, name=f"xT{c}") for c in range(NCH)]
        wqk_c = [persist.tile([P, 2 * HID], bf16, name=f"wqk{c}") for c in range(NCH)]
        # per-block wv tiles: [p, kc*128+col], contiguous 2KB DMA lines
        wv_blk = [persist.tile([P, NCH * P], bf16, name=f"wv{b}") for b in range(NCH)]
        qT_sb = persist.tile([P, NCH, S], bf16)  # [p, hidout_chunk, seq]
        # K^T stored zero-padded to K=128 per head: variant v holds head
        # 2c+v's 64 rows at partition offset 64*v, the other half zero.
        kTp_sb = persist.tile([P, NCH, 2, S], bf16)
        v_sb = persist.tile([P, NKT, VROW], bf16)  # [p(seq), seq_chunk, 16*(64+1)]
        bqkm_sb = persist.tile([P, 3 * NCH], fp32)  # cols: bq | bk | mask
        bv_sb = persist.tile([P, HID], fp32, name="bv_sb") if has_bv else None
        wscr = persist.tile([P, 512], bf16, name="warm_scratch")

        # ---- input DMAs ----
        # All issued from the Scalar sequencer (idle until the first exp,
        # which the PE-bound pipeline doesn't need early) in strict
        # priority order: with a single issuer the transfers also COMPLETE
        # in priority order at full aggregate DMA bandwidth. v2 spread the
        # issues across three sequencers and the concurrent streams split
        # the 16 DMA engines round-robin -- xT crawled at ~57GB/s and the
        # fill ran to 40us at half clock.
        # NOTE: the 16 DMA engines round-robin descriptors across ALL
        # pending transfers, so a lone transfer only reaches ~128GB/s and
        # full aggregate (~344GB/s) needs several in flight. Equal-size
        # transfers issued in order also COMPLETE in order. So: flood the
        # critical set (everything the fill + first slot needs) first,
        # then flood the rest -- the second wave only competes with the
        # critical set's tail.
        # All input DMAs on the Sync queue: its issues are credit-gated, so
        # transfers complete FIFO in issue order at good per-transfer pace.
        # (Scalar-queue issues are ungated -- everything floods at once and
        # the 16 DMA engines round-robin ALL pending transfers, so the
        # critical xT stream crawls. Two-queue splits hit cross-queue
        # credit contention. Measured: this ordering is the fastest.)
        wqk_half = lambda c, h: (wqk_c[c][:, h * HID:(h + 1) * HID],
                                 wqk[c * P:(c + 1) * P, h * HID:(h + 1) * HID])
        sync_q = [(bqkm_sb, bqkm[:, :]), wqk_half(0, 0), wqk_half(0, 1)]
        for c in range(NCH):
            sync_q.append((xT_c[c], xT[c * P:(c + 1) * P, :]))
        for o, i in sync_q:
            nc.sync.dma_start(out=o, in_=i)
        # gate the remaining input DMAs on xT7's arrival: a tiny DVE copy
        # reading xT_c[7] and writing wqk_c[1] makes the next DMA (and
        # everything queued behind it on sync) wait, keeping the critical
        # 2.6MB exclusive in the DMA engines' round-robin
        nc.vector.tensor_copy(out=wqk_c[1][0:1, 0:2], in_=xT_c[NCH - 1][0:1, 0:2])
        sync_q2 = [wqk_half(1, 0), wqk_half(1, 1)]
        if has_bv:
            bv_bcast = bass.AP(tensor=bv.tensor if hasattr(bv, "tensor") else bv,
                               offset=0, ap=[[0, P], [1, HID]])
            sync_q2.append((bv_sb, bv_bcast))
        sync_q2 += [(wv_blk[0], wvs[0]), (wv_blk[1], wvs[1])]
        for c in range(2, NCH):
            sync_q2 += [wqk_half(c, 0), wqk_half(c, 1), (wv_blk[c], wvs[c])]
        for o, i in sync_q2:
            nc.sync.dma_start(out=o, in_=i)
        # GpSimd: zero-pad + ones-column memsets only; chunk-0 first (its
        # K bias write lands ~14us in), the rest chunk-paced.
        nc.gpsimd.memset(kTp_sb[64:P, 0, 0, :], 0.0)
        nc.gpsimd.memset(kTp_sb[0:64, 0, 1, :], 0.0)
        ones_view = v_sb.rearrange("p st (h x) -> p st h x", x=HD + 1)[:, :, :, HD:HD + 1]
        nc.gpsimd.memset(ones_view, 1.0)
        for c in range(1, NCH):
            nc.gpsimd.memset(kTp_sb[64:P, c, 0, :], 0.0)
            nc.gpsimd.memset(kTp_sb[0:64, c, 1, :], 0.0)

        nc.vector.memset(wscr, 0.5)

        def dummy_mms(n):
            for _ in range(n):
                wps = sc_ps.tile([P, S], fp32, name="score_psum")
                nc.tensor.matmul(
                    wps[:, 0:512],
                    lhsT=wscr[:, 0:P],
                    rhs=wscr,
                    start=True,
                    stop=True,
                )

        # bridge PE from engine start (~6.8us) to first xT arrival
        # (~13us) with UNBROKEN work: the HAM clock-gate ramps to 8/8
        # only after sustained activity, and a single multi-us idle gap
        # here restarts the ramp, leaving everything before ~37us at
        # half clock (measured).
        dummy_mms(14)

        # ---- fill: chunk-0 Q and K projections paced by xT chunk DMAs ----
        # Q halves accumulate in the qkv pool, K halves in the (otherwise
        # idle) cx pool: 4 concurrently-open PSUM chains, 4 banks.
        q_ps = [qkv_ps.tile([P, 512], fp32, name="qkv_psum") for _ in range(2)]
        k_ps = [cx_ps.tile([P, 512], fp32, name="ctx_psum") for _ in range(2)]
        for kc in range(NCH):
            st, sp = (kc == 0), (kc == NCH - 1)
            for half in range(2):
                nc.tensor.matmul(
                    q_ps[half],
                    lhsT=wqk_c[kc][:, 0:P],
                    rhs=xT_c[kc][:, half * 512:(half + 1) * 512],
                    start=st,
                    stop=sp,
                )
                nc.tensor.matmul(
                    k_ps[half],
                    lhsT=wqk_c[kc][:, HID:HID + P],
                    rhs=xT_c[kc][:, half * 512:(half + 1) * 512],
                    start=st,
                    stop=sp,
                )
            # absorb xT arrival jitter so the HAM clock-gate never sees a
            # long idle window during the fill
            dummy_mms(2 if kc < NCH - 1 else 0)
        for half in range(2):
            nc.vector.tensor_scalar_add(
                out=qT_sb[:, 0, half * 512:(half + 1) * 512],
                in0=q_ps[half],
                scalar1=bqkm_sb[:, 0:1],
            )
            for sub in range(2):
                po = 64 * sub
                nc.vector.tensor_scalar_add(
                    out=kTp_sb[po:po + HD, 0, sub, half * 512:(half + 1) * 512],
                    in0=k_ps[half][po:po + HD, :],
                    scalar1=bqkm_sb[po:po + HD, NCH:NCH + 1],
                )

        # ---- emission helpers (each returns filler items: (rows, fn)) ----
        def qk_group(c, half, which):
            def fn():
                ps = qkv_ps.tile([P, 512], fp32, name="qkv_psum")
                off = 0 if which == "q" else HID
                for kc in range(NCH):
                    nc.tensor.matmul(
                        ps,
                        lhsT=wqk_c[kc][:, off + c * P:off + (c + 1) * P],
                        rhs=xT_c[kc][:, half * 512:(half + 1) * 512],
                        start=(kc == 0),
                        stop=(kc == NCH - 1),
                    )
                if which == "q":
                    nc.vector.tensor_scalar_add(
                        out=qT_sb[:, c, half * 512:(half + 1) * 512],
                        in0=ps,
                        scalar1=bqkm_sb[:, c:c + 1],
                    )
                else:
                    for sub in range(2):
                        po = 64 * sub
                        nc.vector.tensor_scalar_add(
                            out=kTp_sb[po:po + HD, c, sub, half * 512:(half + 1) * 512],
                            in0=ps[po:po + HD, :],
                            scalar1=bqkm_sb[po:po + HD, NCH + c:NCH + c + 1],
                        )
            return (8 * 512, fn)

        def vb_chain(blk, st):
            def fn():
                ps = qkv_ps.tile([P, P], fp32, name="qkv_psum")
                for kc in range(NCH):
                    nc.tensor.matmul(
                        ps,
                        lhsT=xT_c[kc][:, st * P:(st + 1) * P],
                        rhs=wv_blk[blk][:, kc * P:(kc + 1) * P],
                        start=(kc == 0),
                        stop=(kc == NCH - 1),
                    )
                dst = (
                    v_sb[:, st, :]
                    .rearrange("p (h x) -> p h x", x=HD + 1)[:, 2 * blk:2 * blk + 2, 0:HD]
                )
                src = ps.rearrange("p (h x) -> p h x", x=HD)
                if has_bv:
                    bvs = (
                        bv_sb[:, 2 * blk * HD:(2 * blk + 2) * HD]
                        .rearrange("p (h x) -> p h x", x=HD)
                    )
                    nc.vector.tensor_add(out=dst, in0=src, in1=bvs)
                else:
                    nc.vector.tensor_copy(out=dst, in_=src)
            return (8 * P, fn)

        pT_tiles = {}
        head_outs = {}

        def ctx_chain(h, qt, qt_dma=False):
            def fn():
                pT_h = pT_tiles[h]
                if qt == 0:
                    head_outs[h] = out_pool.tile([P, NQT, HD], fp32, name="head_out")
                head_out = head_outs[h]
                cps = cx_ps.tile([P, HD + 1], fp32, name="ctx_psum")
                # rotate each chain's accumulation order by qt so chain qt's
                # LAST matmul depends on exp(kt=(qt+7)%8), not the final exp
                for j in range(NKT):
                    kc = (qt + j) % NKT
                    nc.tensor.matmul(
                        cps,
                        lhsT=pT_h[:, kc, qt * P:(qt + 1) * P],
                        rhs=v_sb[:, kc, h * (HD + 1):(h + 1) * (HD + 1)],
                        start=(j == 0),
                        stop=(j == NKT - 1),
                    )
                recip = misc.tile([P, 1], fp32, name="recip")
                nc.vector.reciprocal(recip, cps[:, HD:HD + 1])
                nc.vector.tensor_scalar_mul(
                    out=head_out[:, qt, :],
                    in0=cps[:, 0:HD],
                    scalar1=recip,
                )
                if qt == NQT - 1:
                    dst = bass.AP(
                        tensor=out.tensor if hasattr(out, "tensor") else out,
                        offset=h * HD,
                        ap=[[HID, P], [P * HID, NQT], [1, HD]],
                    )
                    # tail heads go out via the scalar queue (idle once the
                    # exps drain); mid-run heads via sync
                    eng = nc.scalar if qt_dma else nc.sync
                    eng.dma_start(out=dst, in_=head_out)
            return (NKT * (HD + 1), fn)

        def score_tile(h, kt):
            c, sub = h // 2, h % 2
            pT_h = pT_tiles[h]
            ps = sc_ps.tile([P, S], fp32, name="score_psum")
            for half in range(2):
                nc.tensor.matmul(
                    ps[:, half * 512:(half + 1) * 512],
                    lhsT=kTp_sb[:, c, sub, kt * P:(kt + 1) * P],
                    rhs=qT_sb[:, c, half * 512:(half + 1) * 512],
                    start=True,
                    stop=True,
                )
            nc.scalar.activation(
                out=pT_h[:, kt, :],
                in_=ps,
                func=EXP,
                bias=bqkm_sb[:, 2 * NCH + kt:2 * NCH + kt + 1],
                scale=SCALE,
            )

        def riffle(*lists):
            items = []
            ls = [list(l) for l in lists]
            while any(ls):
                for l in ls:
                    if l:
                        items.append(l.pop(0))
            return items

        def emit_slot(c, fillers):
            a, b = 2 * c, 2 * c + 1
            pT_tiles[a] = pT_pool.tile([P, NKT, S], bf16, name="pT")
            pT_tiles[b] = pT_pool.tile([P, NKT, S], bf16, name="pT")
            total = sum(r for r, _ in fillers)
            drained, idx = 0, 0
            for kt in range(NKT):
                score_tile(a, kt)
                score_tile(b, kt)
                quota = total * (kt + 1) // NKT
                while drained < quota and idx < len(fillers):
                    r, fn = fillers[idx]
                    fn()
                    drained += r
                    idx += 1
            while idx < len(fillers):
                fillers[idx][1]()
                idx += 1

        # ---- slots 0..5 ----
        for c in range(NCH - 2):
            qk_items = [qk_group(c + 1, half, w) for w in ("q", "k")
                        for half in range(2)]
            if c == 0:
                vb_items = [vb_chain(0, st) for st in range(NKT)] + \
                           [vb_chain(1, st) for st in range(NKT)]
                # wv blocks 0/1 land after the critical DMA wave; QK(1)
                # first keeps the PE off them until ~mid-slot
                emit_slot(c, qk_items + vb_items)
            else:
                vb_items = [vb_chain(c + 1, st) for st in range(NKT)]
                ctx_a = [ctx_chain(2 * c - 2, qt) for qt in range(NQT)]
                ctx_b = [ctx_chain(2 * c - 1, qt) for qt in range(NQT)]
                emit_slot(c, riffle(qk_items, vb_items, ctx_a, ctx_b))

        # ---- slot 6: heads 12,13 paired + head 14's score burst ----
        # Pulling scores(14) forward makes the final slot score-light, so
        # the last exps drain while ctx(12..14) still feeds the PE and
        # only ctx(15)'s stragglers trail the very last exp.
        qk_items = [qk_group(7, half, w) for w in ("q", "k") for half in range(2)]
        vb_items = [vb_chain(7, st) for st in range(NKT)]
        ctx_a = [ctx_chain(10, qt) for qt in range(NQT)]
        ctx_b = [ctx_chain(11, qt) for qt in range(NQT)]
        emit_slot(6, riffle(qk_items, vb_items, ctx_a, ctx_b))
        pT_tiles[14] = pT_pool.tile([P, NKT, S], bf16, name="pT")
        for kt in range(NKT):
            score_tile(14, kt)

        # ---- slot 7: head 15's burst with ctx(12..14) interleaved ----
        # The burst's matmuls are exp-paced (sc pool bufs); weaving the
        # already-runnable ctx chains between them keeps the PE busy
        # through the final exp drain, leaving only ctx(15)'s stragglers
        # after the very last exp.
        pT_tiles[15] = pT_pool.tile([P, NKT, S], bf16, name="pT")
        tail_ctx = riffle([ctx_chain(12, qt) for qt in range(NQT)],
                          [ctx_chain(13, qt) for qt in range(NQT)],
                          [ctx_chain(14, qt, qt_dma=True) for qt in range(NQT)])
        ti = 0
        for kt in range(NKT):
            score_tile(15, kt)
            take = (len(tail_ctx) * (kt + 1)) // NKT
            while ti < take:
                tail_ctx[ti][1]()
                ti += 1
        for qt in range(NQT):
            ctx_chain(15, qt, qt_dma=True)[1]()

    nc.finalize()
    return nc


def _prep_inputs(inputs):
    bf16 = ml_dtypes.bfloat16
    hs = np.asarray(inputs["hidden_states"], dtype=np.float32)
    am = np.asarray(inputs["attention_mask"], dtype=np.float32)
    Wq = np.asarray(inputs["Wq"], dtype=np.float32)
    Wk = np.asarray(inputs["Wk"], dtype=np.float32)
    Wv = np.asarray(inputs["Wv"], dtype=np.float32)
    bq = np.asarray(inputs["bq"], dtype=np.float32)
    bk = np.asarray(inputs["bk"], dtype=np.float32)
    bv = np.asarray(inputs["bv"], dtype=np.float32)

    has_bv = bool(np.any(bv))

    wq_b = Wq.astype(bf16)
    wk_b = Wk.astype(bf16)
    wv_b = Wv.astype(bf16)
    # wq+wk interleaved per 128-row chunk: [HID, 2*HID]
    wqk = np.ascontiguousarray(
        np.concatenate(
            [wq_b.reshape(NCH, P, HID), wk_b.reshape(NCH, P, HID)], axis=2
        ).reshape(HID, 2 * HID)
    )
    # wv as [head-pair block][partition][chunk][col]
    wvs = np.ascontiguousarray(
        wv_b.reshape(NCH, P, NCH, P).transpose(2, 1, 0, 3)
    )
    bq_c = bq.reshape(NCH, P).T
    bk_c = bk.reshape(NCH, P).T

    hs_b = hs.astype(bf16)
    in_maps = []
    for b in range(B):
        mask_c = am[b, 0, 0].reshape(NKT, P).T
        m = {
            "xT": np.ascontiguousarray(hs_b[b].T),
            "wqk": wqk,
            "wvs": wvs,
            "bqkm": np.ascontiguousarray(
                np.concatenate([bq_c, bk_c, mask_c], axis=1)
            ),
        }
        if has_bv:
            m["bv"] = bv
        in_maps.append(m)
    return in_maps, has_bv


def _run(inputs, trace=False, trace_cores=None):
    from concourse.bass_utils import run_bass_kernel_spmd

    in_maps, has_bv = _prep_inputs(inputs)
    nc = _build(has_bv)
    res = run_bass_kernel_spmd(
        nc, in_maps, core_ids=list(range(N_CORES)), trace=trace,
        trace_cores=trace_cores,
    )
    out = np.stack([np.asarray(r["out"], dtype=np.float32) for r in res.results])
    return out, res


def kernel(**inputs) -> np.ndarray:
    out, _ = _run(inputs, trace=False)
    return out


# revision 36
# speedup vs baseline: 1.1894x; 1.1894x over previous
"""BERT self-attention on 8 Trainium2 NeuronCores.

Sharding: data-parallel over batch (B=8 -> 1 batch element per core).
Every core runs the same single-core Bass kernel on its own batch slice;
weights/mask are replicated. The final output is a host-side stack.

Per-core algorithm (S=1024, HID=1024, NH=16, HD=64), all matmuls bf16
with fp32 PSUM accumulation:

  xT = X^T (host-transposed, bf16)             [HID, S]
  Q^T = Wq^T @ X^T   (lhsT = Wq natural)       [HID, S]  (+bq per-partition)
  K^T = Wk^T @ X^T                             [HID, S]  (+bk per-partition)
  V   = X @ Wv       (lhsT = xT)               [S, HID]  (+bv broadcast; see below)
  per head h:
    S^T = K_h @ Q_h^T            (scoresT: [k, q]; K^T stored zero-padded to
                                  128 contraction rows so FWL stays enabled)
    P^T = exp(S^T/8 + mask[k])   (ScalarE, mask is per-partition in this layout,
                                  max-subtraction skipped: |scores/8| <~ 4)
    ctx = P^T.T @ [V_h + bv | 1] (lhsT = P^T directly, no transposes anywhere;
                                  the ones column yields the softmax denominator Z)
    out[:, h] = ctx[:, :64] * (1/Z)   (== softmax(S) @ (V+bv) = attn + bv)

The ScalarE exp stream (~147us) is the critical path; the serial score
matmuls keep ScalarE lagging slightly so it never gaps. Changes vs the
original baseline: more warmup matmuls (the HAM clock-gate dropped to
4/8 mid-fill otherwise), anti-throttle matmuls before the last head's
ctx (the tail ran at half clock), and each head's output leaves as ONE
3D-AP DMA instead of eight serialized ~0.6us transfers.
"""

import functools

import numpy as np
import ml_dtypes

B, S, HID = 8, 1024, 1024
NH, HD = 16, 64
P = 128
NCH = HID // P  # hid chunks (8)
NKT = S // P  # key tiles (8)
NQT = S // P  # query tiles (8)
VROW = NH * (HD + 1)  # 1040: per-seq-chunk V row: 16 x (64 V cols + ones col)
N_CORES = 8

SCALE = 1.0 / float(np.sqrt(HD))


@functools.lru_cache(maxsize=None)
def _build(has_bv: bool):
    import concourse.bass as bass
    import concourse.tile as tile
    from concourse import bacc, mybir
    from contextlib import ExitStack

    fp32 = mybir.dt.float32
    bf16 = mybir.dt.bfloat16
    EXP = mybir.ActivationFunctionType.Exp

    nc = bacc.Bacc("TRN2", target_bir_lowering=False)

    xT = nc.dram_tensor("xT", [HID, S], bf16, kind="ExternalInput")
    wq = nc.dram_tensor("wq", [HID, HID], bf16, kind="ExternalInput")
    wk = nc.dram_tensor("wk", [HID, HID], bf16, kind="ExternalInput")
    wv = nc.dram_tensor("wv", [HID, HID], bf16, kind="ExternalInput")
    bq = nc.dram_tensor("bq", [P, NCH], fp32, kind="ExternalInput")
    bk = nc.dram_tensor("bk", [P, NCH], fp32, kind="ExternalInput")
    bv = nc.dram_tensor("bv", [HID], fp32, kind="ExternalInput") if has_bv else None
    mask = nc.dram_tensor("mask", [P, NKT], fp32, kind="ExternalInput")
    out = nc.dram_tensor("out", [S, HID], fp32, kind="ExternalOutput")

    with tile.TileContext(nc) as tc, ExitStack() as ctx:
        persist = ctx.enter_context(tc.tile_pool(name="persist", bufs=1))
        misc = ctx.enter_context(tc.tile_pool(name="misc", bufs=8))
        pT_pool = ctx.enter_context(tc.tile_pool(name="pT", bufs=3))
        out_pool = ctx.enter_context(tc.tile_pool(name="out", bufs=2))
        qkv_ps = ctx.enter_context(tc.tile_pool(name="qkv_ps", bufs=2, space="PSUM"))
        sc_ps = ctx.enter_context(tc.tile_pool(name="sc_ps", bufs=2, space="PSUM"))
        cx_ps = ctx.enter_context(tc.tile_pool(name="cx_ps", bufs=2, space="PSUM"))

        # ---- persistent SBUF tensors ----
        # per-chunk tiles: a matmul touching chunk kc then only depends on
        # that one chunk's DMA, so PE work starts ~2 chunks into the fill
        xT_c = [persist.tile([P, S], bf16, name=f"xT{c}") for c in range(NCH)]
        wq_c = [persist.tile([P, HID], bf16, name=f"wq{c}") for c in range(NCH)]
        wk_c = [persist.tile([P, HID], bf16, name=f"wk{c}") for c in range(NCH)]
        wv_c = [persist.tile([P, HID], bf16, name=f"wv{c}") for c in range(NCH)]
        qT_sb = persist.tile([P, NCH, S], bf16)  # [p, hidout_chunk, seq]
        # K^T stored zero-padded to K=128 per head: variant v holds head
        # 2c+v's 64 rows at partition offset 64*v, the other half zero.
        # This keeps the score matmuls at 128 contraction rows (FWL stays
        # enabled; 64-row weight loads serialize ~100ns/matmul otherwise).
        kTp_sb = persist.tile([P, NCH, 2, S], bf16)
        v_sb = persist.tile([P, NKT, VROW], bf16)  # [p(seq), seq_chunk, 16*(64+1)]
        bq_sb = persist.tile([P, NCH], fp32)
        bk_sb = persist.tile([P, NCH], fp32)
        mask_sb = persist.tile([P, NKT], fp32)
        premask_sb = persist.tile([P, NKT], fp32, name="premask")
        bv_sb = persist.tile([P, HID], fp32, name="bv_sb") if has_bv else None

        # ---- input DMAs ----
        nc.sync.dma_start(out=bq_sb, in_=bq[:, :])
        nc.sync.dma_start(out=bk_sb, in_=bk[:, :])
        nc.sync.dma_start(out=mask_sb, in_=mask[:, :])
        if has_bv:
            # broadcast bv[HID] across all 128 partitions
            bv_bcast = bass.AP(tensor=bv.tensor if hasattr(bv, "tensor") else bv,
                               offset=0, ap=[[0, P], [1, HID]])
            nc.sync.dma_start(out=bv_sb, in_=bv_bcast)
        for c in range(NCH):
            nc.sync.dma_start(out=xT_c[c], in_=xT[c * P:(c + 1) * P, :])
            nc.sync.dma_start(out=wv_c[c], in_=wv[c * P:(c + 1) * P, :])
        for c in range(NCH):
            nc.sync.dma_start(out=wq_c[c], in_=wq[c * P:(c + 1) * P, :])
            nc.sync.dma_start(out=wk_c[c], in_=wk[c * P:(c + 1) * P, :])

        # ones columns for the softmax denominator live at col 64 of each
        # 65-wide head block; V copies below only overwrite cols 0..63
        nc.gpsimd.memset(v_sb, 1.0)
        # zero the padded K^T store on the otherwise-idle gpsimd engine;
        # the K copies later fill in only each variant's live 64 rows
        nc.gpsimd.memset(kTp_sb, 0.0)

        # warmup matmuls on scratch data while the input DMAs stream in:
        # keeps the PE busy so the HAM clock-gate reaches 8/8 before real
        # work arrives and stays there through the DMA-paced V ramp
        # (otherwise the clock drops to 4/8 mid-fill for ~14us).
        wscr = persist.tile([P, 512], bf16, name="warm_scratch")
        nc.vector.memset(wscr, 0.5)
        # fast-exp per-partition bias: (s + 8*mask + 704) * 16/ln2 ==
        # (s/8 + mask) * 128/ln2 + 16250.5, the bf16-bits exp argument
        nc.vector.tensor_scalar(
            out=premask_sb,
            in0=mask_sb,
            scalar1=8.0,
            scalar2=704.0,
            op0=mybir.AluOpType.mult,
            op1=mybir.AluOpType.add,
        )

        def dummy_mms(n):
            for _ in range(n):
                wps = sc_ps.tile([P, S], fp32, name="score_psum")
                nc.tensor.matmul(
                    wps[:, 0:512],
                    lhsT=wscr[:, 0:P],
                    rhs=wscr,
                    start=True,
                    stop=True,
                )

        dummy_mms(28)

        # ---- emission helpers ----
        def v_group(st, half):
            ps = qkv_ps.tile([P, 512], fp32, name="qkv_psum")
            for kc in range(NCH):
                nc.tensor.matmul(
                    ps,
                    lhsT=xT_c[kc][:, st * P:(st + 1) * P],
                    rhs=wv_c[kc][:, half * 512:(half + 1) * 512],
                    start=(kc == 0),
                    stop=(kc == NCH - 1),
                )
            dst = (
                v_sb[:, st, :]
                .rearrange("p (h x) -> p h x", x=HD + 1)[:, half * 8:(half + 1) * 8, 0:HD]
            )
            src = ps.rearrange("p (h x) -> p h x", x=HD)
            if has_bv:
                bvs = (
                    bv_sb[:, half * 512:(half + 1) * 512]
                    .rearrange("p (h x) -> p h x", x=HD)
                )
                nc.vector.tensor_add(out=dst, in0=src, in1=bvs)
            else:
                nc.vector.tensor_copy(out=dst, in_=src)

        def qk_proj(c, bridges=False):
            # bridges: for chunk 0 the projections are DMA-gated (wq/wk
            # stream in 17-28us); dummy matmuls between the groups keep the
            # PE from idling past the HAM threshold (some runs dropped to
            # half clock here, a stochastic ~3-7us penalty)
            for half in range(2):
                ps = qkv_ps.tile([P, 512], fp32, name="qkv_psum")
                for kc in range(NCH):
                    nc.tensor.matmul(
                        ps,
                        lhsT=wq_c[kc][:, c * P:(c + 1) * P],
                        rhs=xT_c[kc][:, half * 512:(half + 1) * 512],
                        start=(kc == 0),
                        stop=(kc == NCH - 1),
                    )
                nc.vector.tensor_scalar_add(
                    out=qT_sb[:, c, half * 512:(half + 1) * 512],
                    in0=ps,
                    scalar1=bq_sb[:, c:c + 1],
                )
                if bridges:
                    dummy_mms(2)
            for half in range(2):
                ps = qkv_ps.tile([P, 512], fp32, name="qkv_psum")
                for kc in range(NCH):
                    nc.tensor.matmul(
                        ps,
                        lhsT=wk_c[kc][:, c * P:(c + 1) * P],
                        rhs=xT_c[kc][:, half * 512:(half + 1) * 512],
                        start=(kc == 0),
                        stop=(kc == NCH - 1),
                    )
                for sub in range(2):  # head 2c+sub lives at partitions 64*sub..
                    po = 64 * sub
                    nc.vector.tensor_scalar_add(
                        out=kTp_sb[po:po + HD, c, sub, half * 512:(half + 1) * 512],
                        in0=ps[po:po + HD, :],
                        scalar1=bk_sb[po:po + HD, c:c + 1],
                    )
                if bridges and half == 0:
                    dummy_mms(2)

        pT_tiles = {}

        # Two of each head's eight exp tiles run on the otherwise-spare
        # DVE via a bf16 Schraudolph bit-trick: bf16_bits(exp(x)) ~=
        # int16((x + m + 704)*16/ln2 ... ) -- i.e. one tensor_scalar of
        # the fp32 scores with an int16 output bitcast as the bf16 P^T
        # slice. Max rel err 3.3% on 2/8 of the keys -> measured final
        # rel_err 8.5e-3 (gate 2e-2). The ScalarE exp stream (the
        # mid-run pacer) drops from 8 to 6 tiles per head; kt=7 on the
        # DVE also unblocks the rotated ctx chains earlier.
        FAST_KT = (3, 7)
        FEXP_C1 = float(16.0 / np.log(2.0))  # 23.0831...; x*c1+c2, c2=704*c1
        int16 = mybir.dt.int16

        def score_head(c, sub):
            pT_h = pT_tiles.get((c, sub))
            if pT_h is None:
                pT_h = pT_pool.tile([P, NKT, S], bf16, name="pT")
                pT_tiles[(c, sub)] = pT_h
            for kt in range(NKT):
                ps = sc_ps.tile([P, S], fp32, name="score_psum")
                for half in range(2):
                    nc.tensor.matmul(
                        ps[:, half * 512:(half + 1) * 512],
                        lhsT=kTp_sb[:, c, sub, kt * P:(kt + 1) * P],
                        rhs=qT_sb[:, c, half * 512:(half + 1) * 512],
                        start=True,
                        stop=True,
                    )
                if kt in FAST_KT:
                    nc.vector.tensor_scalar(
                        out=pT_h[:, kt, :].bitcast(int16),
                        in0=ps,
                        scalar1=premask_sb[:, kt:kt + 1],
                        scalar2=FEXP_C1,
                        op0=mybir.AluOpType.add,
                        op1=mybir.AluOpType.mult,
                    )
                else:
                    # P^T = exp(scores/8 + mask_k); bf16 out, straight to SBUF
                    nc.scalar.activation(
                        out=pT_h[:, kt, :],
                        in_=ps,
                        func=EXP,
                        bias=mask_sb[:, kt:kt + 1],
                        scale=SCALE,
                    )

        # ---- prologue: start the exp stream ~23us earlier by weaving
        # QK(c0)+scores(c0) into the V phase (wq/wk land ~28us; V's tail
        # and QK(c1) then fill the exp-paced score stalls via the PE's
        # reorder window) ----
        V_ORDER = [(st, half) for st in range(NKT) for half in range(2)]
        for i, (st, half) in enumerate(V_ORDER[0:8]):
            v_group(st, half)
            if i < 6:
                # bridge the DMA-paced gaps between early V groups so the
                # HAM clock-gate never sees a >3us idle window (it dropped
                # to 4/8 for ~7us mid-fill otherwise)
                dummy_mms(2)
        qk_proj(0, bridges=True)
        score_head(0, 0)
        for st, half in V_ORDER[8:14]:
            v_group(st, half)
        score_head(0, 1)
        for st, half in V_ORDER[14:20]:
            v_group(st, half)
        qk_proj(1)
        score_head(1, 0)
        for st, half in V_ORDER[20:32]:
            v_group(st, half)

        # ---- steady chunks ----
        for c in range(NCH):
            if c >= 2:
                qk_proj(c)
            if c >= 2:
                score_head(c, 0)
            if c >= 1:
                score_head(c, 1)

            for sub in range(2):
                h = 2 * c + sub
                pT_h = pT_tiles[(c, sub)]
                head_out = out_pool.tile([P, NQT, HD], fp32, name="head_out")
                if c == NCH - 1 and sub == 1:
                    # keep the PE clock at 8/8 through the final exps so
                    # the last head's ctx doesn't run at half speed
                    # (3 suffice now: the rotated ctx chains below are
                    # themselves runnable during the exp drain and keep
                    # the PE active; more dummies just delay them)
                    dummy_mms(3)
                for qt in range(NQT):
                    cps = cx_ps.tile([P, HD + 1], fp32, name="ctx_psum")
                    # rotate each chain's accumulation order by qt: the
                    # chain's LAST matmul then depends on exp(kt=(qt+7)%8)
                    # instead of every chain waiting the chunk's final exp
                    # -- 7 of 8 chains complete before the last exp lands,
                    # cutting the after-last-exp tail from ~4.4us to ~1us
                    for j in range(NKT):
                        kc = (qt + j) % NKT
                        nc.tensor.matmul(
                            cps,
                            lhsT=pT_h[:, kc, qt * P:(qt + 1) * P],
                            rhs=v_sb[:, kc, h * (HD + 1):(h + 1) * (HD + 1)],
                            start=(j == 0),
                            stop=(j == NKT - 1),
                        )
                    recip = misc.tile([P, 1], fp32, name="recip")
                    nc.vector.reciprocal(recip, cps[:, HD:HD + 1])
                    nc.vector.tensor_scalar_mul(
                        out=head_out[:, qt, :],
                        in0=cps[:, 0:HD],
                        scalar1=recip,
                    )
                # one 3D-AP DMA for the whole head's [S, 64] output block
                # (the old per-qt pattern cost 8 serialized ~0.6us DMAs,
                # a ~5us tail after the final exps)
                dst = bass.AP(
                    tensor=out.tensor if hasattr(out, "tensor") else out,
                    offset=h * HD,
                    ap=[[HID, P], [P * HID, NQT], [1, HD]],
                )
                nc.sync.dma_start(out=dst, in_=head_out)

    nc.finalize()
    return nc


def _prep_inputs(inputs):
    bf16 = ml_dtypes.bfloat16
    hs = np.asarray(inputs["hidden_states"], dtype=np.float32)
    am = np.asarray(inputs["attention_mask"], dtype=np.float32)
    Wq = np.asarray(inputs["Wq"], dtype=np.float32)
    Wk = np.asarray(inputs["Wk"], dtype=np.float32)
    Wv = np.asarray(inputs["Wv"], dtype=np.float32)
    bq = np.asarray(inputs["bq"], dtype=np.float32)
    bk = np.asarray(inputs["bk"], dtype=np.float32)
    bv = np.asarray(inputs["bv"], dtype=np.float32)

    has_bv = bool(np.any(bv))

    wq_b = np.ascontiguousarray(Wq.astype(bf16))
    wk_b = np.ascontiguousarray(Wk.astype(bf16))
    wv_b = np.ascontiguousarray(Wv.astype(bf16))
    bq_c = np.ascontiguousarray(bq.reshape(NCH, P).T)
    bk_c = np.ascontiguousarray(bk.reshape(NCH, P).T)

    hs_b = hs.astype(bf16)
    in_maps = []
    for b in range(B):
        m = {
            "xT": np.ascontiguousarray(hs_b[b].T),
            "wq": wq_b,
            "wk": wk_b,
            "wv": wv_b,
            "bq": bq_c,
            "bk": bk_c,
            "mask": np.ascontiguousarray(am[b, 0, 0].reshape(NKT, P).T),
        }
        if has_bv:
            m["bv"] = bv
        in_maps.append(m)
    return in_maps, has_bv


def _run(inputs, trace=False, trace_cores=None):
    from concourse.bass_utils import run_bass_kernel_spmd

    in_maps, has_bv = _prep_inputs(inputs)
    nc = _build(has_bv)
    res = run_bass_kernel_spmd(
        nc, in_maps, core_ids=list(range(N_CORES)), trace=trace,
        trace_cores=trace_cores,
    )
    out = np.stack([np.asarray(r["out"], dtype=np.float32) for r in res.results])
    return out, res


def kernel(**inputs) -> np.ndarray:
    out, _ = _run(inputs, trace=False)
    return out

